# revision 1
# baseline (speedup 1.0000x reference)
"""Trainium2 Bass kernel for nn_Decoder (mapping MLP + hard-LSTM scan + out proj).

Self-contained: takes FULL inputs (as produced by setup_inputs), shards batch
across 8 NeuronCores, runs a Bass/Tile kernel via run_bass_kernel_spmd, and
gathers the full [T, K, B, C] output.

Layout per core (B' = B/8 = 512 batch elems):
  rows = k*B' + b  (20 "rtiles" of 512 rows each, one per k)
  h, c state: [H=128 partitions, 512 rows] tiles, one pair per rtile.
  Per step & rtile: PSUM gates via W_ih·x preload (4 row-packed contract-2
  matmuls) + W_hh·h accumulation (4 bf16 matmuls); custom DVE ops fuse
  hardsigmoid/hardtanh + multiply reading gates straight from PSUM; one
  vector add for c; out-projection matmul reuses a consumed PSUM slice and
  DMAs straight to DRAM.
"""
import os
import sys

sys.path.insert(0, "/opt/trn_rl_repo")

import numpy as np
import ml_dtypes
from contextlib import ExitStack

import concourse.bass as bass
import concourse.tile as tile
from concourse import mybir, bacc
import concourse.dve_ops as _dve_ops_mod
from concourse.dve_ops import DveOp, OPS, CUSTOM_DVE_SPECS, _CUSTOM_DVE_ROW_BASE
from concourse.dve_spec import (
    Spec, Src0, Src1, C0, C1, C2, Zero, One, maxx, minn, relu, lower, _has_src1,
)
from concourse.dve_uop import DveOpSpec
from concourse.bass_utils import run_bass_kernel_spmd

FP32 = mybir.dt.float32
BF16 = mybir.dt.bfloat16
AF = mybir.ActivationFunctionType

# Full-problem config (hardcoded; the harness always calls with these shapes).
T_FULL, K_FULL, B_FULL, C_DIM, H_DIM, MH_DIM, N_CORES = 20, 20, 4096, 2, 128, 64, 8


# ---------------------------------------------------------------- custom ops
def _register_op(name, spec):
    for op in OPS:
        if op.name == name:
            return op
    shas = {}
    for ver in ("v3", "v4"):
        tmp = DveOpSpec(name=name, opcode=0, uops=lower(spec, ver=ver),
                        rd1_en=_has_src1(spec))
        shas[ver] = tmp.sha(ver)
    op = DveOp(name, spec, subdim=False, uops_sha=shas)
    OPS.append(op)
    CUSTOM_DVE_SPECS[name] = spec
    _dve_ops_mod._SUB_OPCODE_FOR_NAME[name] = _CUSTOM_DVE_ROW_BASE + len(OPS) - 1
    assert _dve_ops_mod._SUB_OPCODE_FOR_NAME[name] < 0x20
    return op


def _hs(x, s0, s1):
    return np.minimum(np.maximum(x * s0 + s1, 0.0), 1.0)


# t = hs(i_psum)*ht(g); in1 = a_g = relu(g+1+bg) from ACT, ht = min(a_g-1, 1)
OP_T = _register_op(
    "ANT_LSTM_T",
    Spec(body=minn(relu(Src0 * C0 + C1), One) * minn(Src1 - One, One),
         reference=lambda in0, in1, s0, s1, imm2:
             _hs(in0, s0, s1) * np.minimum(in1 - 1.0, 1.0)),
)
# u = hs(f_psum)*c
OP_U = _register_op(
    "ANT_LSTM_U",
    Spec(body=minn(relu(Src0 * C0 + C1), One) * Src1,
         reference=lambda in0, in1, s0, s1, imm2: _hs(in0, s0, s1) * in1),
)
# h = min(a_o,1)*clip(c, -1, 1); in0 = a_o = relu(o/6+hsb_o) from ACT; imm2=-1
OP_H = _register_op(
    "ANT_LSTM_H",
    Spec(body=minn(Src0, One) * maxx(minn(Src1, One), C2),
         reference=lambda in0, in1, s0, s1, imm2:
             np.minimum(in0, 1.0) * np.maximum(np.minimum(in1, 1.0), imm2)),
)
# leaky_relu(psum + bm1) = max(y, 0.01*y), y = Src0 + C1; imm2 = slope
OP_LRELU = _register_op(
    "ANT_LRELU",
    Spec(body=maxx(Src0 + C1, (Src0 + C1) * C2),
         reference=lambda in0, in1, s0, s1, imm2:
             np.maximum(in0 + s1, (in0 + s1) * imm2)),
)


# ---------------------------------------------------------------- bass build
def build_nc(T, K, BP, use_pack=True, add_eng="vector", ut_dt=None,
             psa_bufs=1, pack_out=True, c_dt=None, repeat=1, tmp_bufs=4,
             c_add_pe=False, reorder=False):
    """Build the per-core Bass program. BP = per-core batch.

    PSUM layout per rtile: psA = [g | o] (freed right after the merged a_go
    ACT evac), psB = [i | f] (read by OP_T / OP_U). Biases + hardsigmoid
    pre-scale folded into the augmented W_ih·[x;1] preload on the host, so
    i/f/o psum arrive as (raw+b)/6+0.5 and g as raw+b+1.
    """
    H, MH, C = H_DIM, MH_DIM, C_DIM
    CA = C + 1        # augmented x rows (x0, x1, 1)
    FD = BP           # free dim of every tile
    PSB = 512         # PSUM bank stride in fp32 elems
    RT = K            # rtiles per step
    UT_DT = ut_dt or FP32
    C_DT = c_dt or FP32
    nc = bacc.Bacc("TRN2", target_bir_lowering=False, debug=False)

    phT_e = nc.declare_dram_parameter("phT", [H, K * BP], BF16, isOutput=False)
    xh_e = nc.declare_dram_parameter("xh", [CA, T * BP], BF16, isOutput=False)
    whhT_e = nc.declare_dram_parameter("whhT", [H, 4 * H], BF16, isOutput=False)
    wihT_e = nc.declare_dram_parameter("wihT", [CA, 4 * H], BF16, isOutput=False)
    woutT_e = nc.declare_dram_parameter("woutT", [H, C], BF16, isOutput=False)
    wm1T_e = nc.declare_dram_parameter("wm1T", [H, MH], BF16, isOutput=False)
    wm2T_e = nc.declare_dram_parameter("wm2T", [MH, H], BF16, isOutput=False)
    biasp_e = nc.declare_dram_parameter("biasp", [H, 2], FP32, isOutput=False)
    ident_e = nc.declare_dram_parameter("ident", [H, H], BF16, isOutput=False)
    out_e = nc.declare_dram_parameter("out", [T, K, C, BP], FP32, isOutput=True)

    with tile.TileContext(nc) as tc:
        with ExitStack() as ctx:
            wts = ctx.enter_context(tc.tile_pool(name="wts", bufs=1))
            big = ctx.enter_context(tc.tile_pool(name="big", bufs=1))
            st = ctx.enter_context(tc.tile_pool(name="st", bufs=1))
            tmp = ctx.enter_context(tc.tile_pool(name="tmp", bufs=tmp_bufs))
            psA = ctx.enter_context(
                tc.tile_pool(name="psA", bufs=psa_bufs, space="PSUM"))
            psB = ctx.enter_context(tc.tile_pool(name="psB", bufs=2, space="PSUM"))
            psO = (ctx.enter_context(tc.tile_pool(name="psO", bufs=1, space="PSUM"))
                   if pack_out else None)

            # ---- weights / constants into SBUF
            whhT = wts.tile([H, 4 * H], BF16, tag="whhT")
            nc.sync.dma_start(whhT[:], whhT_e[:])
            woutT = wts.tile([H, C], BF16, tag="woutT")
            nc.sync.dma_start(woutT[:], woutT_e[:])
            wm1T = wts.tile([H, MH], BF16, tag="wm1T")
            nc.sync.dma_start(wm1T[:], wm1T_e[:])
            wm2T = wts.tile([MH, H], BF16, tag="wm2T")
            nc.sync.dma_start(wm2T[:], wm2T_e[:])
            biasp = wts.tile([H, 2], FP32, tag="biasp")
            nc.sync.dma_start(biasp[:], biasp_e[:])
            if c_add_pe:
                ident = wts.tile([H, H], BF16, tag="ident")
                nc.sync.dma_start(ident[:], ident_e[:])

            npack = 4 if use_pack else 1
            nprows = 32 * (npack - 1) + CA
            wih = wts.tile([nprows, 4 * H], BF16, tag="wih")
            xrep = wts.tile([nprows, T * BP], BF16, tag="xrep")
            for r in range(npack):
                nc.sync.dma_start(wih[32 * r:32 * r + CA, :], wihT_e[:])
                nc.sync.dma_start(xrep[32 * r:32 * r + CA, :], xh_e[:])

            phT = big.tile([H, K * BP], BF16, tag="phT")
            nc.sync.dma_start(phT[:], phT_e[:])

            # persistent col-packed out psum tiles, zeroed once so the
            # unwritten partition rows stay clean for the batched evac
            if pack_out:
                po_t = [psO.tile([98, PSB], FP32, name=f"po{q}", tag=f"po{q}")
                        for q in range(2)]
                for q in range(2):
                    nc.vector.memset(po_t[q][:], 0.0)

            # ---- persistent state tiles
            h_t = [st.tile([H, FD], BF16, name=f"h{j}", tag=f"h{j}")
                   for j in range(RT)]
            c_t = [st.tile([H, FD], C_DT, name=f"c{j}", tag=f"c{j}")
                   for j in range(RT)]

            # ---- mapping MLP -> h0
            for j in range(RT):
                pa = psA.tile([H, 2 * PSB], FP32, tag="pa")
                nc.tensor.matmul(pa[0:MH, 0:FD], wm1T[:, 0:MH],
                                 phT[:, j * FD:(j + 1) * FD],
                                 start=True, stop=True)
                a1 = tmp.tile([MH, FD], BF16, tag="a1")
                nc.vector._custom_dve(OP_LRELU, out=a1[:], in0=pa[0:MH, 0:FD],
                                      s1=biasp[0:MH, 0:1], imm2=0.01)
                nc.tensor.matmul(pa[0:H, PSB:PSB + FD], wm2T[:, 0:H], a1[:],
                                 start=True, stop=True)
                nc.scalar.activation(h_t[j][:], pa[0:H, PSB:PSB + FD],
                                     AF.Identity, bias=biasp[:, 1:2], scale=1.0)

            # gate chunk offsets in whhT / wih cols: i=0, f=1, g=2, o=3
            CH = {"i": 0, "f": 1, "g": 2, "o": 3}

            def gcol(name):
                m = CH[name]
                return slice(m * H, (m + 1) * H)

            # ---- time loop (repeat>1 is for timing only)
            for t in range(T * repeat):
                t = t % T
                xcols = slice(t * BP, (t + 1) * BP)
                for j in range(RT):
                    pa = psA.tile([H, 2 * PSB], FP32, tag="pa")  # [g | o]
                    pb = psB.tile([H, 2 * PSB], FP32, tag="pb")  # [i | f]
                    if pack_out and j % 4 == 0:
                        po = po_t[(j // 4) % 2]
                    dsts = [(pa, slice(0, FD), "g"),
                            (pa, slice(PSB, PSB + FD), "o"),
                            (pb, slice(0, FD), "i"),
                            (pb, slice(PSB, PSB + FD), "f")]
                    # W_ih·[x;1] preload (contract=3), row-packed across the PE
                    def gi_mm(sel):
                        for r, (ps, sl, gname) in enumerate(dsts):
                            if gname not in sel:
                                continue
                            rr = r if use_pack else 0
                            nc.tensor.matmul(
                                ps[:, sl],
                                wih[32 * rr:32 * rr + CA, gcol(gname)],
                                xrep[32 * rr:32 * rr + CA, xcols],
                                start=True, stop=False,
                                tile_position=(32 * rr, 0) if use_pack else None,
                            )
                    def gh_mm(sel):
                        for ps, sl, gname in dsts:
                            if gname in sel:
                                nc.tensor.matmul(ps[:, sl], whhT[:, gcol(gname)],
                                                 h_t[j][:], start=False,
                                                 stop=True)
                    if not reorder:
                        gi_mm("gofi")
                        gh_mm("gofi")

                    if reorder:
                        gi_mm("go")
                        gh_mm("go")
                    # ACT: merged a_go = relu([g|o] psum) (scale/bias prefolded)
                    a_go = tmp.tile([H, 2 * PSB], BF16, tag="ago")
                    if FD == PSB:
                        nc.scalar.activation(a_go[:], pa[:, 0:2 * PSB], AF.Relu,
                                             bias=0.0, scale=1.0)
                    else:  # small-config fallback: banks not fully written
                        nc.scalar.activation(a_go[:, 0:FD], pa[:, 0:FD],
                                             AF.Relu, bias=0.0, scale=1.0)
                        nc.scalar.activation(a_go[:, PSB:PSB + FD],
                                             pa[:, PSB:PSB + FD],
                                             AF.Relu, bias=0.0, scale=1.0)
                    a_g = a_go[:, 0:FD]
                    a_o = a_go[:, PSB:PSB + FD]
                    if reorder:
                        gi_mm("if")
                        gh_mm("if")

                    # DVE: t = hs(i)*ht(g)
                    if t == 0:
                        t_dst = c_t[j]      # c0 = 0 -> c1 = t
                    else:
                        t_dst = tmp.tile([H, FD], UT_DT, tag="tt")
                    nc.vector._custom_dve(OP_T, out=t_dst[:], in0=pb[:, 0:FD],
                                          in1=a_g, s0=1.0, s1=0.0)
                    if t > 0:
                        u_d = tmp.tile([H, FD], UT_DT, tag="uu")
                        nc.vector._custom_dve(OP_U, out=u_d[:],
                                              in0=pb[:, PSB:PSB + FD],
                                              in1=c_t[j][:], s0=1.0, s1=0.0)
                        if c_add_pe:
                            # c = u + t on the PE (identity-matmul accumulate
                            # into psB's consumed i-slice), ACT evacuates
                            nc.tensor.matmul(pb[:, 0:FD], ident[:], u_d[:],
                                             start=True, stop=False)
                            nc.tensor.matmul(pb[:, 0:FD], ident[:], t_dst[:],
                                             start=False, stop=True)
                            nc.scalar.activation(c_t[j][:], pb[:, 0:FD],
                                                 AF.Copy, bias=0.0, scale=1.0)
                        else:
                            add_e = getattr(nc, add_eng)
                            add_e.tensor_add(c_t[j][:], u_d[:], t_dst[:])
                    # DVE: h = min(a_o,1)*clip(c,-1,1)
                    nc.vector._custom_dve(OP_H, out=h_t[j][:], in0=a_o,
                                          in1=c_t[j][:], imm2=-1.0)

                    # out projection + evac + DMA
                    if pack_out:
                        rr = j % 4
                        nc.tensor.matmul(po[32 * rr:32 * rr + C, 0:FD],
                                         woutT[:, 0:C], h_t[j][:],
                                         start=True, stop=True,
                                         tile_position=(0, 32 * rr))
                        if rr == 3:
                            out_sb = tmp.tile([98, FD], FP32, tag="osb")
                            nc.scalar.activation(out_sb[:], po[0:98, 0:FD],
                                                 AF.Copy, bias=0.0, scale=1.0)
                            for q in range(4):
                                nc.sync.dma_start(
                                    out_e[t, j - 3 + q],
                                    out_sb[32 * q:32 * q + C, :])
                    else:
                        nc.tensor.matmul(pb[0:C, 0:FD], woutT[:, 0:C], h_t[j][:],
                                         start=True, stop=True)
                        out_sb = tmp.tile([C, FD], FP32, tag="osb")
                        nc.scalar.activation(out_sb[:], pb[0:C, 0:FD], AF.Copy,
                                             bias=0.0, scale=1.0)
                        nc.sync.dma_start(out_e[t, j], out_sb[:])

    nc.finalize()
    return nc


# ---------------------------------------------------------------- host side
def _bf16(x):
    return np.ascontiguousarray(x, dtype=np.float32).astype(ml_dtypes.bfloat16)


def prep_core_inputs(inputs, core, T, K, BP):
    H, MH, C = H_DIM, MH_DIM, C_DIM
    b0 = core * BP
    ph = np.asarray(inputs["pred_lstm_hidden"], np.float32)[:, b0:b0 + BP, :]
    phT = ph.transpose(2, 0, 1).reshape(H, K * BP)
    idx = np.concatenate([[0], np.arange(T - 1)])
    obs = np.asarray(inputs["obs_traj_rel"], np.float32)
    xs = obs[idx][:, b0:b0 + BP, :C]
    xh = xs.transpose(2, 0, 1).reshape(C, T * BP)
    xh = np.concatenate([xh, np.ones((1, T * BP), np.float32)], axis=0)
    bsum = (np.asarray(inputs["b_ih"], np.float32)
            + np.asarray(inputs["b_hh"], np.float32))
    # per-gate-chunk scale and bias folded into W_hh / W_ih / the x=1 row:
    #   i, f, o chunks: psum = (raw + b)/6 + 0.5 ; g chunk: psum = raw + b + 1
    scale = np.ones(4 * H, np.float32) / 6.0
    scale[2 * H:3 * H] = 1.0
    bias_row = bsum * scale
    bias_row[0:2 * H] += 0.5
    bias_row[2 * H:3 * H] += 1.0
    bias_row[3 * H:4 * H] += 0.5
    whh_s = np.asarray(inputs["W_hh"], np.float32) * scale[:, None]
    wih_s = np.asarray(inputs["W_ih"], np.float32) * scale[:, None]
    wih_aug = np.concatenate([wih_s.T, bias_row[None, :]], axis=0)  # [3, 4H]
    biasp = np.zeros((H, 2), np.float32)
    biasp[0:MH, 0] = np.asarray(inputs["bm1"], np.float32)
    biasp[:, 1] = np.asarray(inputs["bm2"], np.float32)
    return {
        "phT": _bf16(phT),
        "xh": _bf16(xh),
        "whhT": _bf16(whh_s.T),
        "wihT": _bf16(wih_aug),
        "woutT": _bf16(np.asarray(inputs["W_out"], np.float32).T),
        "wm1T": _bf16(np.asarray(inputs["Wm1"], np.float32).T),
        "wm2T": _bf16(np.asarray(inputs["Wm2"], np.float32).T),
        "biasp": biasp,
        "ident": np.eye(H, dtype=np.float32).astype(ml_dtypes.bfloat16),
    }


_NC_CACHE = {}


def _get_nc(T, K, BP):
    key = (T, K, BP)
    if key not in _NC_CACHE:
        _NC_CACHE[key] = build_nc(T, K, BP, ut_dt=BF16, c_dt=BF16,
                                  c_add_pe=True)
    return _NC_CACHE[key]


def kernel(**inputs) -> np.ndarray:
    T, K, B, C = T_FULL, K_FULL, B_FULL, C_DIM
    BP = B // N_CORES
    nc = _get_nc(T, K, BP)
    in_maps = [prep_core_inputs(inputs, c, T, K, BP) for c in range(N_CORES)]
    trace = bool(int(os.environ.get("KERNEL_TRACE", "0")))
    res = run_bass_kernel_spmd(nc, in_maps, list(range(N_CORES)), trace=trace)
    if trace:
        kernel.last_exec_time_ns = res.exec_time_ns
        kernel.last_results = res
    # per-core out: [T, K, C, BP] -> full [T, K, B, C]
    full = np.concatenate([res.results[c]["out"] for c in range(N_CORES)], axis=3)
    full = full.transpose(0, 1, 3, 2)
    b_out = np.asarray(inputs["b_out"], np.float32)
    return np.ascontiguousarray(full + b_out, dtype=np.float32)

